# revision 29
# baseline (speedup 1.0000x reference)
"""Trainium2 Bass kernel for nn_ConnectivityGraphGenerator (v2).

Data-parallel over batch B=128: 16 graphs per core on 8 NeuronCores.

Structure (per core, per graph g, N=64 nodes, d=OUT=128 head features):
  1. y = x @ W_gnn            (PE, fp32r, 256-col)           [node, H]
  2. hT = relu(lts^T y + b)   (PE + Pool bias/relu evac)     [H, node]
  3. Z = [A;B | C;D | wa;wb]  (PE: head projections, biases folded via a
     ones-row matmul; node-basis stacked src(0:64)/dst(64:128))  [128, 257]
  4. M = A_i + B_j, P = C_i + D_j over all i<j pair slots via ONE matmul
     per 512-slot block against a constant 0/1 "perm" matrix whose column
     s has ones at rows i(s) and 64+j(s).                     [d, slots]
  5. Y = exp(gam*P + del) in-place in PSUM (ACT), then a single custom
     DVE op computes Q = M^2 * (((Y+a)Y+b)Y+c)  ~= M^2/softplus(P)
     (cubic-in-exp minimax fit, max rel err 8.3e-4 over the data's P
     range; the fit's global scale is folded into del/a/b/c so the
     reduction constant is exactly -1/256).
  6. S[pair] = -1/256 * sum_d Q via per-128-pair-chunk PE matmuls with a
     bf16 negones rhs, accumulated as columns of a [128,16] PSUM tile ->
     S lands PAIRS-MAJOR, so the whole tail runs on [128, 256] tiles.
  7. w-head: ww column of Z gathered into WW[128, G]; one matmul per
     chunk (lhsT=perm chunk, rhs=WW) gives wa_i+wb_j for all graphs.
  8. tail: sim=exp(S), w=sigmoid(W), ez=exp(2w)/ln(u)^2, v=sim*ez.
     Host divides by the global sum of ez (softmax couples all cores)
     and scatters into the dense adjacency.
"""

import math

import numpy as np

import concourse.bacc as bacc
import concourse.mybir as mybir
import concourse.tile as tile
from concourse.bass_utils import run_bass_kernel_spmd

F32 = mybir.dt.float32
F32R = mybir.dt.float32r
BF16 = mybir.dt.bfloat16
AF = mybir.ActivationFunctionType
ALU = mybir.AluOpType

B, N, T = 128, 64, 256
IN, H, OUT = N + T, 256, 128
E = N * (N - 1) // 2  # 2016
NCORES = 8
G = B // NCORES  # 16 graphs per core
SLOTS = 2048  # padded pair slots per graph (16 chunks of 128)
NCH = SLOTS // 128  # 16
QW = 512  # pair-slot block width (one PSUM bank)
NQ = SLOTS // QW  # 4

# cubic-in-exp 1/softplus fit: R(p) ~= s*(((Y+a)Y+b)Y+c), Y=exp(g*p+d)
_FG = -0.3259735585994775
_FD = 0.679605110572354
_FA = -1.0432340615452715
_FB = 2.6298350796477363
_FC = 0.27253610703689884
_FS = 0.1588914927761679
# fold s into the exp bias / coefficients: Y' = s^(1/3) * Y
_SIG = _FS ** (1.0 / 3.0)
EXP_SCALE = _FG
EXP_BIAS = _FD + math.log(_FS) / 3.0
CUB_A = _FA * _SIG
CUB_B = _FB * _SIG ** 2
CUB_C = _FC * _SIG ** 3
RED_SCALE = -0.5 / OUT  # exactly -1/256, bf16-representable

# ---------------------------------------------------------------- custom op
_QCUBE = None


def _ref_qcube(in0, in1, c0, c1, c2):
    m2 = in0.astype(np.float32) ** 2
    y = in1.astype(np.float32)
    return m2 * (((y + c0) * y + c1) * y + c2)


def _get_qcube():
    global _QCUBE
    if _QCUBE is not None:
        return _QCUBE
    import concourse.dve_ops as dve_ops
    from concourse.dve_spec import Spec, Src0, Src1, C0, C1, C2, sq, lower

    name = "QCUBE_ANT"
    body = sq(Src0) * (((Src1 + C0) * Src1 + C1) * Src1 + C2)
    spec = Spec(body=body, reference=_ref_qcube)
    if name not in dve_ops._SUB_OPCODE_FOR_NAME:
        row = max(dve_ops._SUB_OPCODE_FOR_NAME.values()) + 1
        assert row < 0x20
        dve_ops._SUB_OPCODE_FOR_NAME[name] = row
    dve_ops.CUSTOM_DVE_SPECS[name] = spec
    shas = {}
    for ver in ("v3", "v4"):
        spec_l = lower(spec, ver=ver)
        tmp = dve_ops.DveOpSpec(
            name=name,
            opcode=dve_ops._SUB_OPCODE_FOR_NAME[name],
            uops=spec_l,
            rd1_en=True,
        )
        shas[ver] = tmp.sha(ver)
    _QCUBE = dve_ops.DveOp(name, spec, subdim=False, uops_sha=shas)
    return _QCUBE


# ---------------------------------------------------------------- device body
def _body(ctx, tc):
    nc = tc.nc
    qcube = _get_qcube()
    r = lambda ap: ap.bitcast(F32R)

    # static-input table column layout (host builds the same order)
    WTAB_COLS = 768 + 514 + 514 + 2 + 64 + 257 + G * NCH
    O_WG, O_WMVT, O_WMVB = 0, 768, 1282
    O_BG, O_LTS, O_BROW = 1796, 1798, 1862
    O_U = 2119

    xt_d = nc.dram_tensor("xt", [128, G, 3, N], F32, kind="ExternalInput").ap()
    wtab_d = nc.dram_tensor("wtab", [128, WTAB_COLS], F32, kind="ExternalInput").ap()
    perm_d = nc.dram_tensor("perm8", [128, SLOTS], mybir.dt.int8, kind="ExternalInput").ap()
    v_d = nc.dram_tensor("v", [128, G * NCH], F32, kind="ExternalOutput").ap()
    ez_d = nc.dram_tensor("ez", [128, G * NCH], F32, kind="ExternalOutput").ap()

    singles = ctx.enter_context(tc.tile_pool(name="singles", bufs=1))
    wtab_t = singles.tile([128, WTAB_COLS], F32)
    xt_all = singles.tile([128, G, 3, N], F32)
    perm8_t = singles.tile([128, SLOTS], mybir.dt.int8)
    NH = 2  # first graphs shipped separately so compute starts early
    nc.sync.dma_start(wtab_t[:, 0:O_U], wtab_d[:, 0:O_U])
    nc.sync.dma_start(xt_all[:, 0:NH], xt_d[:, 0:NH])
    nc.sync.dma_start(perm8_t[:], perm_d[:])
    nc.sync.dma_start(xt_all[:, NH:G], xt_d[:, NH:G])
    nc.sync.dma_start(wtab_t[:, O_U:], wtab_d[:, O_U:])

    wg_t = wtab_t[:, O_WG : O_WG + 768].rearrange("p (c k) -> p c k", c=3)
    wmvt_t = wtab_t[:, O_WMVT : O_WMVT + 514].rearrange("p (c k) -> p c k", c=2)
    wmvb_t = wtab_t[:, O_WMVB : O_WMVB + 514].rearrange("p (c k) -> p c k", c=2)
    bg_t = wtab_t[:, O_BG : O_BG + 2]
    lts_t = wtab_t[0:64, O_LTS : O_LTS + 64]
    brow_t = wtab_t[0:1, O_BROW : O_BROW + 257]
    u_t = wtab_t[:, O_U : O_U + G * NCH]
    perm_sb = singles.tile([128, SLOTS], F32)
    nc.gpsimd.tensor_copy(perm_sb[:], perm8_t[:])
    perm_t = perm_sb[:]

    ones64 = singles.tile([1, 64], F32)
    nc.vector.memset(ones64[:], 1.0)
    negq = singles.tile([128, 1], BF16)
    nc.vector.memset(negq[:], RED_SCALE)
    ebias = singles.tile([128, 1], F32)
    nc.vector.memset(ebias[:], EXP_BIAS)
    ww_t = singles.tile([128, G], F32)
    s_all = singles.tile([128, G * NCH], F32)
    # gumbel prefactor 1/ln(u)^2 early, while the exp/ln ACT table is live
    gu_t = singles.tile([128, G * NCH], F32)
    nc.scalar.activation(gu_t[:], u_t[:], AF.Ln)
    nc.vector.tensor_mul(gu_t[:], gu_t[:], gu_t[:])
    nc.vector.reciprocal_approx_fast(gu_t[:], gu_t[:])

    ys = ctx.enter_context(tc.tile_pool(name="ys", bufs=3))
    hts = ctx.enter_context(tc.tile_pool(name="hts", bufs=3))
    zs = ctx.enter_context(tc.tile_pool(name="zs", bufs=3))
    qs = ctx.enter_context(tc.tile_pool(name="qs", bufs=8))
    tails = ctx.enter_context(tc.tile_pool(name="tails", bufs=1))
    psA = ctx.enter_context(tc.tile_pool(name="psA", bufs=6, space="PSUM"))
    psB = ctx.enter_context(tc.tile_pool(name="psB", bufs=1, space="PSUM"))
    psC = ctx.enter_context(tc.tile_pool(name="psC", bufs=1, space="PSUM"))

    for g in range(G):
        fr = psB.tile([128, 384], F32, tag="frz")
        # y = x @ W_gnn  -> fr[0:64, 0:256]
        for c in range(3):
            nc.tensor.matmul(
                fr[0:64, 0:256],
                lhsT=r(xt_all[:, g, c, :]),
                rhs=r(wg_t[:, c, :]),
                start=(c == 0),
                stop=(c == 2),
            )
        y_sb = ys.tile([64, 256], F32, tag="y")
        nc.gpsimd.tensor_copy(y_sb[:], fr[0:64, 0:256])
        # hT chunks -> fr[:, 256:384]
        for c in range(2):
            nc.tensor.matmul(
                fr[:, 256 + 64 * c : 320 + 64 * c],
                lhsT=r(y_sb[:, 128 * c : 128 * c + 128]),
                rhs=r(lts_t[:]),
                start=True,
                stop=True,
            )
        ht_t = hts.tile([128, 2, N], F32, tag="ht")
        for c in range(2):
            nc.gpsimd.tensor_scalar(
                out=ht_t[:, c, :],
                in0=fr[:, 256 + 64 * c : 320 + 64 * c],
                scalar1=bg_t[:, c : c + 1],
                scalar2=0.0,
                op0=ALU.add,
                op1=ALU.max,
            )
        # heads: Z = [A;B | C;D | wa;wb]
        z_ps = psB.tile([128, 384], F32, tag="frz")
        for c in range(2):
            nc.tensor.matmul(
                z_ps[0:64, 0:257],
                lhsT=r(ht_t[:, c, :]),
                rhs=r(wmvt_t[:, c, :]),
                start=(c == 0),
                stop=(c == 1),
            )
        for c in range(2):
            nc.tensor.matmul(
                z_ps[64:128, 0:257],
                lhsT=r(ht_t[:, c, :]),
                rhs=r(wmvb_t[:, c, :]),
                start=(c == 0),
                stop=False,
            )
        nc.tensor.matmul(
            z_ps[64:128, 0:257], lhsT=r(ones64[:]), rhs=r(brow_t[:]), start=False, stop=True
        )
        z_sb = zs.tile([128, 257], F32, tag="z")
        nc.gpsimd.tensor_copy(z_sb[:], z_ps[:, 0:257])
        nc.gpsimd.tensor_copy(ww_t[:, g : g + 1], z_sb[:, 256:257])

        q_tiles = []
        for q in range(NQ):
            m_ps = psA.tile([128, QW], F32, tag="mp")
            p_ps = psA.tile([128, QW], F32, tag="mp")
            nc.tensor.matmul(
                m_ps[:],
                lhsT=r(z_sb[:, 0:128]),
                rhs=r(perm_t[:, QW * q : QW * q + QW]),
                start=True,
                stop=True,
            )
            nc.tensor.matmul(
                p_ps[:],
                lhsT=r(z_sb[:, 128:256]),
                rhs=r(perm_t[:, QW * q : QW * q + QW]),
                start=True,
                stop=True,
            )
            nc.scalar.activation(p_ps[:], p_ps[:], AF.Exp, bias=ebias[:], scale=EXP_SCALE)
            q_sb = qs.tile([128, QW], BF16, tag="q")
            nc.vector._custom_dve(
                qcube, out=q_sb[:], in0=m_ps[:], in1=p_ps[:],
                s0=CUB_A, s1=CUB_B, imm2=CUB_C,
            )
            q_tiles.append(q_sb)
        # deferred d-reductions: issued after all qcubes so PE.SEQ never
        # head-of-line blocks the next quarter's M/P matmuls
        s_ps = psC.tile([128, 256], F32, tag="sw")
        for q in range(NQ):
            for c4 in range(QW // 128):
                ch = (QW // 128) * q + c4
                nc.tensor.matmul(
                    s_ps[:, ch : ch + 1],
                    lhsT=q_tiles[q][:, 128 * c4 : 128 * c4 + 128],
                    rhs=negq[:],
                    start=(ch == 0),
                    stop=(ch == NCH - 1),
                )
        nc.gpsimd.tensor_copy(s_all[:, NCH * g : NCH * g + NCH], s_ps[:, 0:NCH])

    # w-head pairs: one matmul per chunk, all graphs at once
    w_ps = psC.tile([128, NCH * G], F32, tag="sw")
    for c in range(NCH):
        nc.tensor.matmul(
            w_ps[:, G * c : G * c + G],
            lhsT=r(perm_t[:, 128 * c : 128 * c + 128]),
            rhs=r(ww_t[:]),
            start=(c == 0),
            stop=(c == NCH - 1),
        )
    # tail on [128, G*NCH] pairs-major tiles
    # sigmoid without the Sigmoid ACT table (stays on the exp/ln set):
    # sg = 1/(1 + exp(-W)) via Exp + DVE add/reciprocal
    w_sb = tails.tile([128, G * NCH], F32)
    w_cg = w_ps[:].rearrange("p (c g) -> p c g", c=NCH)
    w_out = w_sb[:].rearrange("p (g c) -> p c g", g=G)
    nc.scalar.activation(w_out, w_cg, AF.Exp, scale=-1.0)
    nc.vector.tensor_scalar_add(w_sb[:], w_sb[:], 1.0)
    nc.vector.reciprocal_approx_fast(w_sb[:], w_sb[:])
    e2w = tails.tile([128, G * NCH], F32)
    nc.scalar.activation(e2w[:], w_sb[:], AF.Exp, scale=2.0)
    sim_t = tails.tile([128, G * NCH], F32)
    nc.scalar.activation(sim_t[:], s_all[:], AF.Exp)
    ez_t = tails.tile([128, G * NCH], F32)
    nc.vector.tensor_mul(ez_t[:], e2w[:], gu_t[:])
    nc.sync.dma_start(ez_d[:], ez_t[:])
    v_t = tails.tile([128, G * NCH], F32)
    nc.vector.tensor_mul(v_t[:], ez_t[:], sim_t[:])
    nc.sync.dma_start(v_d[:], v_t[:])


_NC_CACHE = None


def _build_nc():
    global _NC_CACHE
    if _NC_CACHE is not None:
        return _NC_CACHE
    from contextlib import ExitStack

    nc = bacc.Bacc(
        "TRN2",
        target_bir_lowering=False,
        debug=False,
        enable_asserts=False,
        num_devices=NCORES,
    )
    with tile.TileContext(nc) as tc, ExitStack() as ctx:
        _body(ctx, tc)
    nc.compile()
    _NC_CACHE = nc
    return nc


def _pair_maps():
    """slot s (0..2015) -> (i, j); device cell = [s % 128, g*16 + s//128]."""
    iu0, iu1 = np.triu_indices(N, k=1)
    return iu0, iu1


def _make_perm():
    iu0, iu1 = _pair_maps()
    perm = np.zeros((128, SLOTS), np.float32)
    s = np.arange(E)
    perm[iu0[s], s] = 1.0
    perm[64 + iu1[s], s] = 1.0
    return perm


def _make_in_maps(
    x_topology, x_temporal, gumbel_u, W_gnn, b_gnn, W_mean, b_mean, W_var, b_var, W_w, b_w
):
    f = np.float32
    x_full = np.concatenate(
        [np.asarray(x_topology, f), np.asarray(x_temporal, f)], axis=-1
    )  # [B, N, IN]
    xT = np.ascontiguousarray(np.swapaxes(x_full, 1, 2))  # [B, IN, N]
    xT_pad = np.zeros((B, 128, 3, N), f)
    xT_pad[:, :, 0, :] = xT[:, 0:128]
    xT_pad[:, :, 1, :] = xT[:, 128:256]
    xT_pad[:, 0:64, 2, :] = xT[:, 256:320]
    # device layout [128, G, 3, N] per core (transpose at core split below)

    wg = np.zeros((3, 128, H), f)
    Wg = np.asarray(W_gnn, f)
    wg[0] = Wg[0:128]
    wg[1] = Wg[128:256]
    wg[2, 0:64] = Wg[256:320]

    bg = np.asarray(b_gnn, f).reshape(2, 128).T.copy()  # [128, 2]

    Wm, Wv, Ww = np.asarray(W_mean, f), np.asarray(W_var, f), np.asarray(W_w, f)
    wmvt = np.zeros((2, 128, 257), f)
    wmvb = np.zeros((2, 128, 257), f)
    for c in range(2):
        top = slice(c * 128, c * 128 + 128)
        bot = slice(H + c * 128, H + c * 128 + 128)
        wmvt[c, :, 0:128] = Wm[top]
        wmvt[c, :, 128:256] = Wv[top]
        wmvt[c, :, 256] = Ww[top, 0]
        wmvb[c, :, 0:128] = Wm[bot]
        wmvb[c, :, 128:256] = Wv[bot]
        wmvb[c, :, 256] = Ww[bot, 0]
    brow = np.zeros((1, 257), f)
    brow[0, 0:128] = np.asarray(b_mean, f)
    brow[0, 128:256] = np.asarray(b_var, f)
    brow[0, 256] = np.asarray(b_w, f).reshape(-1)[0]

    j = np.arange(N)
    lts = ((np.arange(N)[:, None] < j[None, :]) / np.maximum(j, 1)[None, :]).astype(f)

    perm = _make_perm()

    # u pairs-major: [128, g*16 + c] = u[slot c*128+p] of graph g
    u_all = np.asarray(gumbel_u, f).reshape(B, E)
    u_dev = np.full((B, 128, NCH), 0.5, f)
    s = np.arange(E)
    u_dev[:, s % 128, s // 128] = u_all[:, s]

    # static-input table, must match the offsets in _body
    wtab = np.zeros((128, 2119 + G * NCH), f)
    wtab[:, 0:768] = np.transpose(wg, (1, 0, 2)).reshape(128, 768)
    wtab[:, 768:1282] = np.transpose(wmvt, (1, 0, 2)).reshape(128, 514)
    wtab[:, 1282:1796] = np.transpose(wmvb, (1, 0, 2)).reshape(128, 514)
    wtab[:, 1796:1798] = bg
    wtab[0:64, 1798:1862] = lts
    wtab[0:1, 1862:2119] = brow
    perm8 = perm.astype(np.int8)

    in_maps = []
    for core in range(NCORES):
        sl = slice(core * G, (core + 1) * G)
        m = {}
        m["xt"] = np.ascontiguousarray(np.transpose(xT_pad[sl], (1, 0, 2, 3)))
        m["perm8"] = perm8
        w = wtab.copy()
        # u: [128, G*NCH] with col = g*NCH + c
        w[:, 2119:] = np.swapaxes(u_dev[sl], 0, 1).reshape(128, G * NCH)
        m["wtab"] = w
        in_maps.append(m)
    return in_maps


def _run_raw(in_maps, trace=False, **kw):
    nc = _build_nc()
    return run_bass_kernel_spmd(
        nc, in_maps, core_ids=list(range(NCORES)), trace=trace, **kw
    )


def kernel(**inputs) -> np.ndarray:
    in_maps = _make_in_maps(**inputs)
    res = _run_raw(in_maps)
    iu0, iu1 = _pair_maps()
    v = np.stack([r["v"] for r in res.results], axis=0)  # [ncores, 128, G*NCH]
    ez = np.stack([r["ez"] for r in res.results], axis=0)
    v4 = v.reshape(NCORES, 128, G, NCH)
    ez4 = ez.reshape(NCORES, 128, G, NCH)
    vals_v = np.transpose(v4, (0, 2, 3, 1)).reshape(NCORES, G, SLOTS)[:, :, :E]
    vals_ez = np.transpose(ez4, (0, 2, 3, 1)).reshape(NCORES, G, SLOTS)[:, :, :E]
    vals_v = vals_v.reshape(B, E)
    vals_ez = vals_ez.reshape(B, E)
    gsum = vals_ez.sum(dtype=np.float32)
    adj = np.zeros((B, N, N), np.float32)
    adj[np.arange(B)[:, None], iu0[None, :], iu1[None, :]] = vals_v / gsum
    return adj


# revision 30
# speedup vs baseline: 1.0661x; 1.0661x over previous
"""Trainium2 Bass kernel for nn_ConnectivityGraphGenerator (v2).

Data-parallel over batch B=128: 16 graphs per core on 8 NeuronCores.

Structure (per core, per graph g, N=64 nodes, d=OUT=128 head features):
  1. y = x @ W_gnn            (PE, fp32r, 256-col)           [node, H]
  2. hT = relu(lts^T y + b)   (PE + Pool bias/relu evac)     [H, node]
  3. Z = [A;B | C;D | wa;wb]  (PE: head projections, biases folded via a
     ones-row matmul; node-basis stacked src(0:64)/dst(64:128))  [128, 257]
  4. M = A_i + B_j, P = C_i + D_j over all i<j pair slots via ONE matmul
     per 512-slot block against a constant 0/1 "perm" matrix whose column
     s has ones at rows i(s) and 64+j(s).                     [d, slots]
  5. Y = exp(gam*P + del) in-place in PSUM (ACT), then a single custom
     DVE op computes Q = M^2 * (((Y+a)Y+b)Y+c)  ~= M^2/softplus(P)
     (cubic-in-exp minimax fit, max rel err 8.3e-4 over the data's P
     range; the fit's global scale is folded into del/a/b/c so the
     reduction constant is exactly -1/256).
  6. S[pair] = -1/256 * sum_d Q via per-128-pair-chunk PE matmuls with a
     bf16 negones rhs, accumulated as columns of a [128,16] PSUM tile ->
     S lands PAIRS-MAJOR, so the whole tail runs on [128, 256] tiles.
  7. w-head: ww column of Z gathered into WW[128, G]; one matmul per
     chunk (lhsT=perm chunk, rhs=WW) gives wa_i+wb_j for all graphs.
  8. tail: sim=exp(S), w=sigmoid(W), ez=exp(2w)/ln(u)^2, v=sim*ez.
     Host divides by the global sum of ez (softmax couples all cores)
     and scatters into the dense adjacency.
"""

import math

import numpy as np

import concourse.bacc as bacc
import concourse.mybir as mybir
import concourse.tile as tile
from concourse.bass_utils import run_bass_kernel_spmd

F32 = mybir.dt.float32
F32R = mybir.dt.float32r
BF16 = mybir.dt.bfloat16
AF = mybir.ActivationFunctionType
ALU = mybir.AluOpType

B, N, T = 128, 64, 256
IN, H, OUT = N + T, 256, 128
E = N * (N - 1) // 2  # 2016
NCORES = 8
G = B // NCORES  # 16 graphs per core
SLOTS = 2048  # padded pair slots per graph (16 chunks of 128)
NCH = SLOTS // 128  # 16
QW = 512  # pair-slot block width (one PSUM bank)
NQ = SLOTS // QW  # 4

# cubic-in-exp 1/softplus fit: R(p) ~= s*(((Y+a)Y+b)Y+c), Y=exp(g*p+d)
_FG = -0.3259735585994775
_FD = 0.679605110572354
_FA = -1.0432340615452715
_FB = 2.6298350796477363
_FC = 0.27253610703689884
_FS = 0.1588914927761679
# fold s into the exp bias / coefficients: Y' = s^(1/3) * Y
_SIG = _FS ** (1.0 / 3.0)
EXP_SCALE = _FG
EXP_BIAS = _FD + math.log(_FS) / 3.0
CUB_A = _FA * _SIG
CUB_B = _FB * _SIG ** 2
CUB_C = _FC * _SIG ** 3
RED_SCALE = -0.5 / OUT  # exactly -1/256, bf16-representable

# ---------------------------------------------------------------- custom op
_QCUBE = None


def _ref_qcube(in0, in1, c0, c1, c2):
    m2 = in0.astype(np.float32) ** 2
    y = in1.astype(np.float32)
    return m2 * (((y + c0) * y + c1) * y + c2)


def _get_qcube():
    global _QCUBE
    if _QCUBE is not None:
        return _QCUBE
    import concourse.dve_ops as dve_ops
    from concourse.dve_spec import Spec, Src0, Src1, C0, C1, C2, sq, lower

    name = "QCUBE_ANT"
    body = sq(Src0) * (((Src1 + C0) * Src1 + C1) * Src1 + C2)
    spec = Spec(body=body, reference=_ref_qcube)
    if name not in dve_ops._SUB_OPCODE_FOR_NAME:
        row = max(dve_ops._SUB_OPCODE_FOR_NAME.values()) + 1
        assert row < 0x20
        dve_ops._SUB_OPCODE_FOR_NAME[name] = row
    dve_ops.CUSTOM_DVE_SPECS[name] = spec
    shas = {}
    for ver in ("v3", "v4"):
        spec_l = lower(spec, ver=ver)
        tmp = dve_ops.DveOpSpec(
            name=name,
            opcode=dve_ops._SUB_OPCODE_FOR_NAME[name],
            uops=spec_l,
            rd1_en=True,
        )
        shas[ver] = tmp.sha(ver)
    _QCUBE = dve_ops.DveOp(name, spec, subdim=False, uops_sha=shas)
    return _QCUBE


# ---------------------------------------------------------------- device body
def _body(ctx, tc):
    nc = tc.nc
    qcube = _get_qcube()
    r = lambda ap: ap.bitcast(F32R)

    # static-input table column layout (host builds the same order)
    WTAB_COLS = 768 + 514 + 514 + 2 + 64 + 257 + G * NCH
    O_WG, O_WMVT, O_WMVB = 0, 768, 1282
    O_BG, O_LTS, O_BROW = 1796, 1798, 1862
    O_U = 2119

    xt_d = nc.dram_tensor("xt", [128, G, 3, N], F32, kind="ExternalInput").ap()
    wtab_d = nc.dram_tensor("wtab", [128, WTAB_COLS], F32, kind="ExternalInput").ap()
    perm_d = nc.dram_tensor("perm8", [128, SLOTS], mybir.dt.int8, kind="ExternalInput").ap()
    v_d = nc.dram_tensor("v", [128, G * NCH], F32, kind="ExternalOutput").ap()
    ez_d = nc.dram_tensor("ez", [128, G * NCH], F32, kind="ExternalOutput").ap()

    singles = ctx.enter_context(tc.tile_pool(name="singles", bufs=1))
    wtab_t = singles.tile([128, WTAB_COLS], F32)
    xt_all = singles.tile([128, G, 3, N], F32)
    perm8_t = singles.tile([128, SLOTS], mybir.dt.int8)
    NH = 2  # first graphs shipped separately so compute starts early
    nc.sync.dma_start(wtab_t[:, 0:O_U], wtab_d[:, 0:O_U])
    nc.sync.dma_start(xt_all[:, 0:NH], xt_d[:, 0:NH])
    nc.sync.dma_start(perm8_t[:], perm_d[:])
    nc.sync.dma_start(xt_all[:, NH:G], xt_d[:, NH:G])
    nc.sync.dma_start(wtab_t[:, O_U:], wtab_d[:, O_U:])

    wg_t = wtab_t[:, O_WG : O_WG + 768].rearrange("p (c k) -> p c k", c=3)
    wmvt_t = wtab_t[:, O_WMVT : O_WMVT + 514].rearrange("p (c k) -> p c k", c=2)
    wmvb_t = wtab_t[:, O_WMVB : O_WMVB + 514].rearrange("p (c k) -> p c k", c=2)
    bg_t = wtab_t[:, O_BG : O_BG + 2]
    lts_t = wtab_t[0:64, O_LTS : O_LTS + 64]
    brow_t = wtab_t[0:1, O_BROW : O_BROW + 257]
    u_t = wtab_t[:, O_U : O_U + G * NCH]
    perm_sb = singles.tile([128, SLOTS], F32)
    nc.gpsimd.tensor_copy(perm_sb[:], perm8_t[:])
    perm_t = perm_sb[:]

    ones64 = singles.tile([1, 64], F32)
    nc.vector.memset(ones64[:], 1.0)
    negq = singles.tile([128, 1], BF16)
    nc.vector.memset(negq[:], RED_SCALE)
    ebias = singles.tile([128, 1], F32)
    nc.vector.memset(ebias[:], EXP_BIAS)
    ww_t = singles.tile([128, G], F32)
    s_all = singles.tile([128, G * NCH], F32)
    # gumbel prefactor 1/ln(u)^2 early, while the exp/ln ACT table is live
    gu_t = singles.tile([128, G * NCH], F32)
    nc.scalar.activation(gu_t[:], u_t[:], AF.Ln)
    nc.vector.tensor_mul(gu_t[:], gu_t[:], gu_t[:])
    nc.vector.reciprocal_approx_fast(gu_t[:], gu_t[:])

    ys = ctx.enter_context(tc.tile_pool(name="ys", bufs=3))
    hts = ctx.enter_context(tc.tile_pool(name="hts", bufs=3))
    zs = ctx.enter_context(tc.tile_pool(name="zs", bufs=3))
    qs = ctx.enter_context(tc.tile_pool(name="qs", bufs=8))
    tails = ctx.enter_context(tc.tile_pool(name="tails", bufs=1))
    psA = ctx.enter_context(tc.tile_pool(name="psA", bufs=5, space="PSUM"))
    psB = ctx.enter_context(tc.tile_pool(name="psB", bufs=2, space="PSUM"))
    psC = ctx.enter_context(tc.tile_pool(name="psC", bufs=1, space="PSUM"))

    for g in range(G):
        fr = psB.tile([128, 384], F32, tag="frz")
        # y = x @ W_gnn  -> fr[0:64, 0:256]
        for c in range(3):
            nc.tensor.matmul(
                fr[0:64, 0:256],
                lhsT=r(xt_all[:, g, c, :]),
                rhs=r(wg_t[:, c, :]),
                start=(c == 0),
                stop=(c == 2),
            )
        y_sb = ys.tile([64, 256], F32, tag="y")
        nc.gpsimd.tensor_copy(y_sb[:], fr[0:64, 0:256])
        # hT chunks -> fr[:, 256:384]
        for c in range(2):
            nc.tensor.matmul(
                fr[:, 256 + 64 * c : 320 + 64 * c],
                lhsT=r(y_sb[:, 128 * c : 128 * c + 128]),
                rhs=r(lts_t[:]),
                start=True,
                stop=True,
            )
        ht_t = hts.tile([128, 2, N], F32, tag="ht")
        for c in range(2):
            nc.gpsimd.tensor_scalar(
                out=ht_t[:, c, :],
                in0=fr[:, 256 + 64 * c : 320 + 64 * c],
                scalar1=bg_t[:, c : c + 1],
                scalar2=0.0,
                op0=ALU.add,
                op1=ALU.max,
            )
        # heads: Z = [A;B | C;D | wa;wb]
        z_ps = psB.tile([128, 384], F32, tag="frz")
        for c in range(2):
            nc.tensor.matmul(
                z_ps[0:64, 0:257],
                lhsT=r(ht_t[:, c, :]),
                rhs=r(wmvt_t[:, c, :]),
                start=(c == 0),
                stop=(c == 1),
            )
        for c in range(2):
            nc.tensor.matmul(
                z_ps[64:128, 0:257],
                lhsT=r(ht_t[:, c, :]),
                rhs=r(wmvb_t[:, c, :]),
                start=(c == 0),
                stop=False,
            )
        nc.tensor.matmul(
            z_ps[64:128, 0:257], lhsT=r(ones64[:]), rhs=r(brow_t[:]), start=False, stop=True
        )
        z_sb = zs.tile([128, 257], F32, tag="z")
        nc.gpsimd.tensor_copy(z_sb[:], z_ps[:, 0:257])
        nc.gpsimd.tensor_copy(ww_t[:, g : g + 1], z_sb[:, 256:257])

        q_tiles = []
        for q in range(NQ):
            m_ps = psA.tile([128, QW], F32, tag="mp")
            p_ps = psA.tile([128, QW], F32, tag="mp")
            nc.tensor.matmul(
                m_ps[:],
                lhsT=r(z_sb[:, 0:128]),
                rhs=r(perm_t[:, QW * q : QW * q + QW]),
                start=True,
                stop=True,
            )
            nc.tensor.matmul(
                p_ps[:],
                lhsT=r(z_sb[:, 128:256]),
                rhs=r(perm_t[:, QW * q : QW * q + QW]),
                start=True,
                stop=True,
            )
            nc.scalar.activation(p_ps[:], p_ps[:], AF.Exp, bias=ebias[:], scale=EXP_SCALE)
            q_sb = qs.tile([128, QW], BF16, tag="q")
            nc.vector._custom_dve(
                qcube, out=q_sb[:], in0=m_ps[:], in1=p_ps[:],
                s0=CUB_A, s1=CUB_B, imm2=CUB_C,
            )
            q_tiles.append(q_sb)
        # deferred d-reductions: issued after all qcubes so PE.SEQ never
        # head-of-line blocks the next quarter's M/P matmuls
        s_ps = psC.tile([128, 256], F32, tag="sw")
        for q in range(NQ):
            for c4 in range(QW // 128):
                ch = (QW // 128) * q + c4
                nc.tensor.matmul(
                    s_ps[:, ch : ch + 1],
                    lhsT=q_tiles[q][:, 128 * c4 : 128 * c4 + 128],
                    rhs=negq[:],
                    start=(ch == 0),
                    stop=(ch == NCH - 1),
                )
        nc.gpsimd.tensor_copy(s_all[:, NCH * g : NCH * g + NCH], s_ps[:, 0:NCH])

    # w-head pairs: one matmul per chunk, all graphs at once
    w_ps = psC.tile([128, NCH * G], F32, tag="sw")
    for c in range(NCH):
        nc.tensor.matmul(
            w_ps[:, G * c : G * c + G],
            lhsT=r(perm_t[:, 128 * c : 128 * c + 128]),
            rhs=r(ww_t[:]),
            start=(c == 0),
            stop=(c == NCH - 1),
        )
    # tail on [128, G*NCH] pairs-major tiles
    # sigmoid without the Sigmoid ACT table (stays on the exp/ln set):
    # sg = 1/(1 + exp(-W)) via Exp + DVE add/reciprocal
    w_sb = tails.tile([128, G * NCH], F32)
    w_cg = w_ps[:].rearrange("p (c g) -> p c g", c=NCH)
    w_out = w_sb[:].rearrange("p (g c) -> p c g", g=G)
    nc.scalar.activation(w_out, w_cg, AF.Exp, scale=-1.0)
    nc.vector.tensor_scalar_add(w_sb[:], w_sb[:], 1.0)
    nc.vector.reciprocal_approx_fast(w_sb[:], w_sb[:])
    e2w = tails.tile([128, G * NCH], F32)
    nc.scalar.activation(e2w[:], w_sb[:], AF.Exp, scale=2.0)
    sim_t = tails.tile([128, G * NCH], F32)
    nc.scalar.activation(sim_t[:], s_all[:], AF.Exp)
    ez_t = tails.tile([128, G * NCH], F32)
    nc.vector.tensor_mul(ez_t[:], e2w[:], gu_t[:])
    nc.sync.dma_start(ez_d[:], ez_t[:])
    v_t = tails.tile([128, G * NCH], F32)
    nc.vector.tensor_mul(v_t[:], ez_t[:], sim_t[:])
    nc.sync.dma_start(v_d[:], v_t[:])


_NC_CACHE = None


def _build_nc():
    global _NC_CACHE
    if _NC_CACHE is not None:
        return _NC_CACHE
    from contextlib import ExitStack

    nc = bacc.Bacc(
        "TRN2",
        target_bir_lowering=False,
        debug=False,
        enable_asserts=False,
        num_devices=NCORES,
    )
    with tile.TileContext(nc) as tc, ExitStack() as ctx:
        _body(ctx, tc)
    nc.compile()
    _NC_CACHE = nc
    return nc


def _pair_maps():
    """slot s (0..2015) -> (i, j); device cell = [s % 128, g*16 + s//128]."""
    iu0, iu1 = np.triu_indices(N, k=1)
    return iu0, iu1


def _make_perm():
    iu0, iu1 = _pair_maps()
    perm = np.zeros((128, SLOTS), np.float32)
    s = np.arange(E)
    perm[iu0[s], s] = 1.0
    perm[64 + iu1[s], s] = 1.0
    return perm


def _make_in_maps(
    x_topology, x_temporal, gumbel_u, W_gnn, b_gnn, W_mean, b_mean, W_var, b_var, W_w, b_w
):
    f = np.float32
    x_full = np.concatenate(
        [np.asarray(x_topology, f), np.asarray(x_temporal, f)], axis=-1
    )  # [B, N, IN]
    xT = np.ascontiguousarray(np.swapaxes(x_full, 1, 2))  # [B, IN, N]
    xT_pad = np.zeros((B, 128, 3, N), f)
    xT_pad[:, :, 0, :] = xT[:, 0:128]
    xT_pad[:, :, 1, :] = xT[:, 128:256]
    xT_pad[:, 0:64, 2, :] = xT[:, 256:320]
    # device layout [128, G, 3, N] per core (transpose at core split below)

    wg = np.zeros((3, 128, H), f)
    Wg = np.asarray(W_gnn, f)
    wg[0] = Wg[0:128]
    wg[1] = Wg[128:256]
    wg[2, 0:64] = Wg[256:320]

    bg = np.asarray(b_gnn, f).reshape(2, 128).T.copy()  # [128, 2]

    Wm, Wv, Ww = np.asarray(W_mean, f), np.asarray(W_var, f), np.asarray(W_w, f)
    wmvt = np.zeros((2, 128, 257), f)
    wmvb = np.zeros((2, 128, 257), f)
    for c in range(2):
        top = slice(c * 128, c * 128 + 128)
        bot = slice(H + c * 128, H + c * 128 + 128)
        wmvt[c, :, 0:128] = Wm[top]
        wmvt[c, :, 128:256] = Wv[top]
        wmvt[c, :, 256] = Ww[top, 0]
        wmvb[c, :, 0:128] = Wm[bot]
        wmvb[c, :, 128:256] = Wv[bot]
        wmvb[c, :, 256] = Ww[bot, 0]
    brow = np.zeros((1, 257), f)
    brow[0, 0:128] = np.asarray(b_mean, f)
    brow[0, 128:256] = np.asarray(b_var, f)
    brow[0, 256] = np.asarray(b_w, f).reshape(-1)[0]

    j = np.arange(N)
    lts = ((np.arange(N)[:, None] < j[None, :]) / np.maximum(j, 1)[None, :]).astype(f)

    perm = _make_perm()

    # u pairs-major: [128, g*16 + c] = u[slot c*128+p] of graph g
    u_all = np.asarray(gumbel_u, f).reshape(B, E)
    u_dev = np.full((B, 128, NCH), 0.5, f)
    s = np.arange(E)
    u_dev[:, s % 128, s // 128] = u_all[:, s]

    # static-input table, must match the offsets in _body
    wtab = np.zeros((128, 2119 + G * NCH), f)
    wtab[:, 0:768] = np.transpose(wg, (1, 0, 2)).reshape(128, 768)
    wtab[:, 768:1282] = np.transpose(wmvt, (1, 0, 2)).reshape(128, 514)
    wtab[:, 1282:1796] = np.transpose(wmvb, (1, 0, 2)).reshape(128, 514)
    wtab[:, 1796:1798] = bg
    wtab[0:64, 1798:1862] = lts
    wtab[0:1, 1862:2119] = brow
    perm8 = perm.astype(np.int8)

    in_maps = []
    for core in range(NCORES):
        sl = slice(core * G, (core + 1) * G)
        m = {}
        m["xt"] = np.ascontiguousarray(np.transpose(xT_pad[sl], (1, 0, 2, 3)))
        m["perm8"] = perm8
        w = wtab.copy()
        # u: [128, G*NCH] with col = g*NCH + c
        w[:, 2119:] = np.swapaxes(u_dev[sl], 0, 1).reshape(128, G * NCH)
        m["wtab"] = w
        in_maps.append(m)
    return in_maps


def _run_raw(in_maps, trace=False, **kw):
    nc = _build_nc()
    return run_bass_kernel_spmd(
        nc, in_maps, core_ids=list(range(NCORES)), trace=trace, **kw
    )


def kernel(**inputs) -> np.ndarray:
    in_maps = _make_in_maps(**inputs)
    res = _run_raw(in_maps)
    iu0, iu1 = _pair_maps()
    v = np.stack([r["v"] for r in res.results], axis=0)  # [ncores, 128, G*NCH]
    ez = np.stack([r["ez"] for r in res.results], axis=0)
    v4 = v.reshape(NCORES, 128, G, NCH)
    ez4 = ez.reshape(NCORES, 128, G, NCH)
    vals_v = np.transpose(v4, (0, 2, 3, 1)).reshape(NCORES, G, SLOTS)[:, :, :E]
    vals_ez = np.transpose(ez4, (0, 2, 3, 1)).reshape(NCORES, G, SLOTS)[:, :, :E]
    vals_v = vals_v.reshape(B, E)
    vals_ez = vals_ez.reshape(B, E)
    gsum = vals_ez.sum(dtype=np.float32)
    adj = np.zeros((B, N, N), np.float32)
    adj[np.arange(B)[:, None], iu0[None, :], iu1[None, :]] = vals_v / gsum
    return adj


# revision 32
# speedup vs baseline: 1.0665x; 1.0004x over previous
"""Trainium2 Bass kernel for nn_ConnectivityGraphGenerator (v2).

Data-parallel over batch B=128: 16 graphs per core on 8 NeuronCores.

Structure (per core, per graph g, N=64 nodes, d=OUT=128 head features):
  1. y = x @ W_gnn            (PE, fp32r, 256-col)           [node, H]
  2. hT = relu(lts^T y + b)   (PE + Pool bias/relu evac)     [H, node]
  3. Z = [A;B | C;D | wa;wb]  (PE: head projections, biases folded via a
     ones-row matmul; node-basis stacked src(0:64)/dst(64:128))  [128, 257]
  4. M = A_i + B_j, P = C_i + D_j over all i<j pair slots via ONE matmul
     per 512-slot block against a constant 0/1 "perm" matrix whose column
     s has ones at rows i(s) and 64+j(s).                     [d, slots]
  5. Y = exp(gam*P + del) in-place in PSUM (ACT), then a single custom
     DVE op computes Q = M^2 * (((Y+a)Y+b)Y+c)  ~= M^2/softplus(P)
     (cubic-in-exp minimax fit, max rel err 8.3e-4 over the data's P
     range; the fit's global scale is folded into del/a/b/c so the
     reduction constant is exactly -1/256).
  6. S[pair] = -1/256 * sum_d Q via per-128-pair-chunk PE matmuls with a
     bf16 negones rhs, accumulated as columns of a [128,16] PSUM tile ->
     S lands PAIRS-MAJOR, so the whole tail runs on [128, 256] tiles.
  7. w-head: ww column of Z gathered into WW[128, G]; one matmul per
     chunk (lhsT=perm chunk, rhs=WW) gives wa_i+wb_j for all graphs.
  8. tail: sim=exp(S), w=sigmoid(W), ez=exp(2w)/ln(u)^2, v=sim*ez.
     Host divides by the global sum of ez (softmax couples all cores)
     and scatters into the dense adjacency.
"""

import math

import numpy as np

import concourse.bacc as bacc
import concourse.mybir as mybir
import concourse.tile as tile
from concourse.bass_utils import run_bass_kernel_spmd

F32 = mybir.dt.float32
F32R = mybir.dt.float32r
BF16 = mybir.dt.bfloat16
AF = mybir.ActivationFunctionType
ALU = mybir.AluOpType

B, N, T = 128, 64, 256
IN, H, OUT = N + T, 256, 128
E = N * (N - 1) // 2  # 2016
NCORES = 8
G = B // NCORES  # 16 graphs per core
SLOTS = 2048  # padded pair slots per graph (16 chunks of 128)
NCH = SLOTS // 128  # 16
QW = 512  # pair-slot block width (one PSUM bank)
NQ = SLOTS // QW  # 4

# cubic-in-exp 1/softplus fit: R(p) ~= s*(((Y+a)Y+b)Y+c), Y=exp(g*p+d)
_FG = -0.3259735585994775
_FD = 0.679605110572354
_FA = -1.0432340615452715
_FB = 2.6298350796477363
_FC = 0.27253610703689884
_FS = 0.1588914927761679
# fold s into the exp bias / coefficients: Y' = s^(1/3) * Y
_SIG = _FS ** (1.0 / 3.0)
EXP_SCALE = _FG
EXP_BIAS = _FD + math.log(_FS) / 3.0
CUB_A = _FA * _SIG
CUB_B = _FB * _SIG ** 2
CUB_C = _FC * _SIG ** 3
RED_SCALE = -0.5 / OUT  # exactly -1/256, bf16-representable

# ---------------------------------------------------------------- custom op
_QCUBE = None


def _ref_qcube(in0, in1, c0, c1, c2):
    m2 = in0.astype(np.float32) ** 2
    y = in1.astype(np.float32)
    return m2 * (((y + c0) * y + c1) * y + c2)


def _get_qcube():
    global _QCUBE
    if _QCUBE is not None:
        return _QCUBE
    import concourse.dve_ops as dve_ops
    from concourse.dve_spec import Spec, Src0, Src1, C0, C1, C2, sq, lower

    name = "QCUBE_ANT"
    body = sq(Src0) * (((Src1 + C0) * Src1 + C1) * Src1 + C2)
    spec = Spec(body=body, reference=_ref_qcube)
    if name not in dve_ops._SUB_OPCODE_FOR_NAME:
        row = max(dve_ops._SUB_OPCODE_FOR_NAME.values()) + 1
        assert row < 0x20
        dve_ops._SUB_OPCODE_FOR_NAME[name] = row
    dve_ops.CUSTOM_DVE_SPECS[name] = spec
    shas = {}
    for ver in ("v3", "v4"):
        spec_l = lower(spec, ver=ver)
        tmp = dve_ops.DveOpSpec(
            name=name,
            opcode=dve_ops._SUB_OPCODE_FOR_NAME[name],
            uops=spec_l,
            rd1_en=True,
        )
        shas[ver] = tmp.sha(ver)
    _QCUBE = dve_ops.DveOp(name, spec, subdim=False, uops_sha=shas)
    return _QCUBE


# ---------------------------------------------------------------- device body
def _body(ctx, tc):
    nc = tc.nc
    qcube = _get_qcube()
    r = lambda ap: ap.bitcast(F32R)

    # static-input table column layout (host builds the same order)
    WTAB_COLS = 768 + 514 + 514 + 2 + 64 + 257 + G * NCH
    O_WG, O_WMVT, O_WMVB = 0, 768, 1282
    O_BG, O_LTS, O_BROW = 1796, 1798, 1862
    O_U = 2119

    xt_d = nc.dram_tensor("xt", [128, G, 3, N], F32, kind="ExternalInput").ap()
    wtab_d = nc.dram_tensor("wtab", [128, WTAB_COLS], F32, kind="ExternalInput").ap()
    perm_d = nc.dram_tensor("perm8", [128, SLOTS], mybir.dt.int8, kind="ExternalInput").ap()
    v_d = nc.dram_tensor("v", [128, G * NCH], F32, kind="ExternalOutput").ap()
    ez_d = nc.dram_tensor("ez", [128, G * NCH], F32, kind="ExternalOutput").ap()

    singles = ctx.enter_context(tc.tile_pool(name="singles", bufs=1))
    wtab_t = singles.tile([128, WTAB_COLS], F32)
    xt_all = singles.tile([128, G, 3, N], F32)
    perm8_t = singles.tile([128, SLOTS], mybir.dt.int8)
    NH = 2  # first graphs shipped separately so compute starts early
    nc.sync.dma_start(wtab_t[:, 0:O_U], wtab_d[:, 0:O_U])
    nc.sync.dma_start(xt_all[:, 0:NH], xt_d[:, 0:NH])
    nc.sync.dma_start(perm8_t[:], perm_d[:])
    nc.sync.dma_start(xt_all[:, NH:G], xt_d[:, NH:G])
    nc.sync.dma_start(wtab_t[:, O_U:], wtab_d[:, O_U:])

    wg_t = wtab_t[:, O_WG : O_WG + 768].rearrange("p (c k) -> p c k", c=3)
    wmvt_t = wtab_t[:, O_WMVT : O_WMVT + 514].rearrange("p (c k) -> p c k", c=2)
    wmvb_t = wtab_t[:, O_WMVB : O_WMVB + 514].rearrange("p (c k) -> p c k", c=2)
    bg_t = wtab_t[:, O_BG : O_BG + 2]
    lts_t = wtab_t[0:64, O_LTS : O_LTS + 64]
    brow_t = wtab_t[0:1, O_BROW : O_BROW + 257]
    u_t = wtab_t[:, O_U : O_U + G * NCH]
    perm_sb = singles.tile([128, SLOTS], F32)
    nc.gpsimd.tensor_copy(perm_sb[:], perm8_t[:])
    perm_t = perm_sb[:]

    ones64 = singles.tile([1, 64], F32)
    nc.vector.memset(ones64[:], 1.0)
    negq = singles.tile([128, 1], BF16)
    nc.vector.memset(negq[:], RED_SCALE)
    ebias = singles.tile([128, 1], F32)
    nc.vector.memset(ebias[:], EXP_BIAS)
    ww_t = singles.tile([128, G], F32)
    s_all = singles.tile([128, G * NCH], F32)
    # gumbel prefactor 1/ln(u)^2 early, while the exp/ln ACT table is live
    gu_t = singles.tile([128, G * NCH], F32)
    nc.scalar.activation(gu_t[:], u_t[:], AF.Ln)
    nc.vector.tensor_mul(gu_t[:], gu_t[:], gu_t[:])
    nc.vector.reciprocal_approx_fast(gu_t[:], gu_t[:])

    ys = ctx.enter_context(tc.tile_pool(name="ys", bufs=3))
    hts = ctx.enter_context(tc.tile_pool(name="hts", bufs=3))
    zs = ctx.enter_context(tc.tile_pool(name="zs", bufs=3))
    qs = ctx.enter_context(tc.tile_pool(name="qs", bufs=8))
    tails = ctx.enter_context(tc.tile_pool(name="tails", bufs=1))
    psA = ctx.enter_context(tc.tile_pool(name="psA", bufs=5, space="PSUM"))
    psB = ctx.enter_context(tc.tile_pool(name="psB", bufs=2, space="PSUM"))
    psC = ctx.enter_context(tc.tile_pool(name="psC", bufs=1, space="PSUM"))

    def emit_reduces(g, q_tiles):
        """d-reduction for graph g, deferred one graph so PE.SEQ never
        head-of-line blocks on the producing qcubes."""
        s_ps = psC.tile([128, 256], F32, tag="sw")
        for q in range(NQ):
            for c4 in range(QW // 128):
                ch = (QW // 128) * q + c4
                nc.tensor.matmul(
                    s_ps[:, ch : ch + 1],
                    lhsT=q_tiles[q][:, 128 * c4 : 128 * c4 + 128],
                    rhs=negq[:],
                    start=(ch == 0),
                    stop=(ch == NCH - 1),
                )
        nc.gpsimd.tensor_copy(s_all[:, NCH * g : NCH * g + NCH], s_ps[:, 0:NCH])

    prev = None  # (g, q_tiles) awaiting reduction
    for g in range(G):
        fr = psB.tile([128, 384], F32, tag="frz")
        # y = x @ W_gnn  -> fr[0:64, 0:256]
        for c in range(3):
            nc.tensor.matmul(
                fr[0:64, 0:256],
                lhsT=r(xt_all[:, g, c, :]),
                rhs=r(wg_t[:, c, :]),
                start=(c == 0),
                stop=(c == 2),
            )
        y_sb = ys.tile([64, 256], F32, tag="y")
        nc.gpsimd.tensor_copy(y_sb[:], fr[0:64, 0:256])
        # hT chunks -> fr[:, 256:384]
        for c in range(2):
            nc.tensor.matmul(
                fr[:, 256 + 64 * c : 320 + 64 * c],
                lhsT=r(y_sb[:, 128 * c : 128 * c + 128]),
                rhs=r(lts_t[:]),
                start=True,
                stop=True,
            )
        ht_t = hts.tile([128, 2, N], F32, tag="ht")
        for c in range(2):
            nc.gpsimd.tensor_scalar(
                out=ht_t[:, c, :],
                in0=fr[:, 256 + 64 * c : 320 + 64 * c],
                scalar1=bg_t[:, c : c + 1],
                scalar2=0.0,
                op0=ALU.add,
                op1=ALU.max,
            )
        # heads: Z = [A;B | C;D | wa;wb]
        z_ps = psB.tile([128, 384], F32, tag="frz")
        for c in range(2):
            nc.tensor.matmul(
                z_ps[0:64, 0:257],
                lhsT=r(ht_t[:, c, :]),
                rhs=r(wmvt_t[:, c, :]),
                start=(c == 0),
                stop=(c == 1),
            )
        for c in range(2):
            nc.tensor.matmul(
                z_ps[64:128, 0:257],
                lhsT=r(ht_t[:, c, :]),
                rhs=r(wmvb_t[:, c, :]),
                start=(c == 0),
                stop=False,
            )
        nc.tensor.matmul(
            z_ps[64:128, 0:257], lhsT=r(ones64[:]), rhs=r(brow_t[:]), start=False, stop=True
        )
        z_sb = zs.tile([128, 257], F32, tag="z")
        nc.gpsimd.tensor_copy(z_sb[:], z_ps[:, 0:257])
        nc.gpsimd.tensor_copy(ww_t[:, g : g + 1], z_sb[:, 256:257])

        q_tiles = []
        for q in range(NQ):
            m_ps = psA.tile([128, QW], F32, tag="mp")
            p_ps = psA.tile([128, QW], F32, tag="mp")
            nc.tensor.matmul(
                m_ps[:],
                lhsT=r(z_sb[:, 0:128]),
                rhs=r(perm_t[:, QW * q : QW * q + QW]),
                start=True,
                stop=True,
            )
            nc.tensor.matmul(
                p_ps[:],
                lhsT=r(z_sb[:, 128:256]),
                rhs=r(perm_t[:, QW * q : QW * q + QW]),
                start=True,
                stop=True,
            )
            nc.scalar.activation(p_ps[:], p_ps[:], AF.Exp, bias=ebias[:], scale=EXP_SCALE)
            q_sb = qs.tile([128, QW], BF16, tag="q")
            nc.vector._custom_dve(
                qcube, out=q_sb[:], in0=m_ps[:], in1=p_ps[:],
                s0=CUB_A, s1=CUB_B, imm2=CUB_C,
            )
            q_tiles.append(q_sb)
        if prev is not None:
            emit_reduces(*prev)
        prev = (g, q_tiles)
    emit_reduces(*prev)

    # w-head pairs: one matmul per chunk, all graphs at once
    w_ps = psC.tile([128, NCH * G], F32, tag="sw")
    for c in range(NCH):
        nc.tensor.matmul(
            w_ps[:, G * c : G * c + G],
            lhsT=r(perm_t[:, 128 * c : 128 * c + 128]),
            rhs=r(ww_t[:]),
            start=(c == 0),
            stop=(c == NCH - 1),
        )
    # tail on [128, G*NCH] pairs-major tiles
    # sigmoid without the Sigmoid ACT table (stays on the exp/ln set):
    # sg = 1/(1 + exp(-W)) via Exp + DVE add/reciprocal
    w_sb = tails.tile([128, G * NCH], F32)
    w_cg = w_ps[:].rearrange("p (c g) -> p c g", c=NCH)
    w_out = w_sb[:].rearrange("p (g c) -> p c g", g=G)
    nc.scalar.activation(w_out, w_cg, AF.Exp, scale=-1.0)
    nc.vector.tensor_scalar_add(w_sb[:], w_sb[:], 1.0)
    nc.vector.reciprocal_approx_fast(w_sb[:], w_sb[:])
    e2w = tails.tile([128, G * NCH], F32)
    nc.scalar.activation(e2w[:], w_sb[:], AF.Exp, scale=2.0)
    sim_t = tails.tile([128, G * NCH], F32)
    nc.scalar.activation(sim_t[:], s_all[:], AF.Exp)
    ez_t = tails.tile([128, G * NCH], F32)
    nc.vector.tensor_mul(ez_t[:], e2w[:], gu_t[:])
    nc.sync.dma_start(ez_d[:], ez_t[:])
    v_t = tails.tile([128, G * NCH], F32)
    nc.vector.tensor_mul(v_t[:], ez_t[:], sim_t[:])
    nc.sync.dma_start(v_d[:], v_t[:])


_NC_CACHE = None


def _build_nc():
    global _NC_CACHE
    if _NC_CACHE is not None:
        return _NC_CACHE
    from contextlib import ExitStack

    nc = bacc.Bacc(
        "TRN2",
        target_bir_lowering=False,
        debug=False,
        enable_asserts=False,
        num_devices=NCORES,
    )
    with tile.TileContext(nc) as tc, ExitStack() as ctx:
        _body(ctx, tc)
    nc.compile()
    _NC_CACHE = nc
    return nc


def _pair_maps():
    """slot s (0..2015) -> (i, j); device cell = [s % 128, g*16 + s//128]."""
    iu0, iu1 = np.triu_indices(N, k=1)
    return iu0, iu1


def _make_perm():
    iu0, iu1 = _pair_maps()
    perm = np.zeros((128, SLOTS), np.float32)
    s = np.arange(E)
    perm[iu0[s], s] = 1.0
    perm[64 + iu1[s], s] = 1.0
    return perm


def _make_in_maps(
    x_topology, x_temporal, gumbel_u, W_gnn, b_gnn, W_mean, b_mean, W_var, b_var, W_w, b_w
):
    f = np.float32
    x_full = np.concatenate(
        [np.asarray(x_topology, f), np.asarray(x_temporal, f)], axis=-1
    )  # [B, N, IN]
    xT = np.ascontiguousarray(np.swapaxes(x_full, 1, 2))  # [B, IN, N]
    xT_pad = np.zeros((B, 128, 3, N), f)
    xT_pad[:, :, 0, :] = xT[:, 0:128]
    xT_pad[:, :, 1, :] = xT[:, 128:256]
    xT_pad[:, 0:64, 2, :] = xT[:, 256:320]
    # device layout [128, G, 3, N] per core (transpose at core split below)

    wg = np.zeros((3, 128, H), f)
    Wg = np.asarray(W_gnn, f)
    wg[0] = Wg[0:128]
    wg[1] = Wg[128:256]
    wg[2, 0:64] = Wg[256:320]

    bg = np.asarray(b_gnn, f).reshape(2, 128).T.copy()  # [128, 2]

    Wm, Wv, Ww = np.asarray(W_mean, f), np.asarray(W_var, f), np.asarray(W_w, f)
    wmvt = np.zeros((2, 128, 257), f)
    wmvb = np.zeros((2, 128, 257), f)
    for c in range(2):
        top = slice(c * 128, c * 128 + 128)
        bot = slice(H + c * 128, H + c * 128 + 128)
        wmvt[c, :, 0:128] = Wm[top]
        wmvt[c, :, 128:256] = Wv[top]
        wmvt[c, :, 256] = Ww[top, 0]
        wmvb[c, :, 0:128] = Wm[bot]
        wmvb[c, :, 128:256] = Wv[bot]
        wmvb[c, :, 256] = Ww[bot, 0]
    brow = np.zeros((1, 257), f)
    brow[0, 0:128] = np.asarray(b_mean, f)
    brow[0, 128:256] = np.asarray(b_var, f)
    brow[0, 256] = np.asarray(b_w, f).reshape(-1)[0]

    j = np.arange(N)
    lts = ((np.arange(N)[:, None] < j[None, :]) / np.maximum(j, 1)[None, :]).astype(f)

    perm = _make_perm()

    # u pairs-major: [128, g*16 + c] = u[slot c*128+p] of graph g
    u_all = np.asarray(gumbel_u, f).reshape(B, E)
    u_dev = np.full((B, 128, NCH), 0.5, f)
    s = np.arange(E)
    u_dev[:, s % 128, s // 128] = u_all[:, s]

    # static-input table, must match the offsets in _body
    wtab = np.zeros((128, 2119 + G * NCH), f)
    wtab[:, 0:768] = np.transpose(wg, (1, 0, 2)).reshape(128, 768)
    wtab[:, 768:1282] = np.transpose(wmvt, (1, 0, 2)).reshape(128, 514)
    wtab[:, 1282:1796] = np.transpose(wmvb, (1, 0, 2)).reshape(128, 514)
    wtab[:, 1796:1798] = bg
    wtab[0:64, 1798:1862] = lts
    wtab[0:1, 1862:2119] = brow
    perm8 = perm.astype(np.int8)

    in_maps = []
    for core in range(NCORES):
        sl = slice(core * G, (core + 1) * G)
        m = {}
        m["xt"] = np.ascontiguousarray(np.transpose(xT_pad[sl], (1, 0, 2, 3)))
        m["perm8"] = perm8
        w = wtab.copy()
        # u: [128, G*NCH] with col = g*NCH + c
        w[:, 2119:] = np.swapaxes(u_dev[sl], 0, 1).reshape(128, G * NCH)
        m["wtab"] = w
        in_maps.append(m)
    return in_maps


def _run_raw(in_maps, trace=False, **kw):
    nc = _build_nc()
    return run_bass_kernel_spmd(
        nc, in_maps, core_ids=list(range(NCORES)), trace=trace, **kw
    )


def kernel(**inputs) -> np.ndarray:
    in_maps = _make_in_maps(**inputs)
    res = _run_raw(in_maps)
    iu0, iu1 = _pair_maps()
    v = np.stack([r["v"] for r in res.results], axis=0)  # [ncores, 128, G*NCH]
    ez = np.stack([r["ez"] for r in res.results], axis=0)
    v4 = v.reshape(NCORES, 128, G, NCH)
    ez4 = ez.reshape(NCORES, 128, G, NCH)
    vals_v = np.transpose(v4, (0, 2, 3, 1)).reshape(NCORES, G, SLOTS)[:, :, :E]
    vals_ez = np.transpose(ez4, (0, 2, 3, 1)).reshape(NCORES, G, SLOTS)[:, :, :E]
    vals_v = vals_v.reshape(B, E)
    vals_ez = vals_ez.reshape(B, E)
    gsum = vals_ez.sum(dtype=np.float32)
    adj = np.zeros((B, N, N), np.float32)
    adj[np.arange(B)[:, None], iu0[None, :], iu1[None, :]] = vals_v / gsum
    return adj
